# revision 11
# baseline (speedup 1.0000x reference)
"""Trainium2 Bass kernel for nn_Extractor_27290222199157.

4-layer pre-norm transformer encoder + VectorQuantize head.
Data-parallel over batch: 8 NeuronCores x 2 batch elements each.
All matmuls fp32 (argmin of the VQ is sensitive to ~1e-4 z perturbations).

Per-core layout:
  - residual h token-major: 4 tiles [128 tok, 512 d] per batch
  - LN: bn_stats mean + centered Square-accum variance + Newton-refined rstd
  - x_hat transposed to feature-major via PE transposes for the matmuls
  - attention: scores.T [k,q] per head (no max-subtract; |s|<2), exp on ACT
    (scale 1/8 fused), A@V with a ones-column appended to V so the softmax
    denominator falls out of the same matmul; normalize via DVE reciprocal +
    gpsimd partition_broadcast
  - VQ: argmax(z . cb^T - |cb|^2/2) via K=65 matmul (ones row on z),
    vector.max + max_index, indirect-DMA codebook gather,
    commit loss via tensor_tensor_reduce, token sums via ones-matmul
"""

import numpy as np

B, S, D = 16, 512, 512
HEADS, DH, DEPTH, FF = 8, 64, 4, 2048
CB_DIM, CB_SIZE = 64, 8192
NCORES = 8
BPC = B // NCORES  # batches per core
P = 128
NC = D // P    # 4 chunks of d
NT = S // P    # 4 chunks of tokens
NF = FF // P   # 16 chunks of ff
NPAIR = HEADS // 2
NK = CB_SIZE // 512  # 16 codebook column chunks

_cached = {}


def _build():
    import os
    STAGE = int(os.environ.get("KSTAGE", "5"))
    KSUB = int(os.environ.get("KSUB", "5"))
    import concourse.bass as bass
    import concourse.tile as tile
    from concourse import bacc, mybir, library_config
    from concourse.masks import make_identity

    F32 = mybir.dt.float32
    U32 = mybir.dt.uint32
    Act = mybir.ActivationFunctionType
    Alu = mybir.AluOpType

    def ts(i, n):
        return slice(i * n, (i + 1) * n)

    nc = bacc.Bacc("TRN2", target_bir_lowering=False, debug=False,
                   num_devices=NCORES)

    xT_d = nc.dram_tensor("xT", [BPC, D, S], F32, kind="ExternalInput")
    w_in_d = nc.dram_tensor("w_in", [D, D], F32, kind="ExternalInput")
    pos_d = nc.dram_tensor("pos_add", [S, D], F32, kind="ExternalInput")
    wq_d = nc.dram_tensor("wq", [DEPTH, D, D], F32, kind="ExternalInput")
    wk_d = nc.dram_tensor("wk", [DEPTH, D, D], F32, kind="ExternalInput")
    wv_d = nc.dram_tensor("wv", [DEPTH, D, D], F32, kind="ExternalInput")
    wo_d = nc.dram_tensor("wo", [DEPTH, D, D], F32, kind="ExternalInput")
    qb_d = nc.dram_tensor("qb", [DEPTH, P, NPAIR], F32, kind="ExternalInput")
    kb_d = nc.dram_tensor("kb", [DEPTH, P, NPAIR], F32, kind="ExternalInput")
    ff1_d = nc.dram_tensor("ff_w1", [DEPTH, D, FF], F32, kind="ExternalInput")
    ff1b_d = nc.dram_tensor("ff1b", [DEPTH, P, NF], F32, kind="ExternalInput")
    ff2_d = nc.dram_tensor("ff_w2", [DEPTH, FF, D], F32, kind="ExternalInput")
    w_out_d = nc.dram_tensor("w_out", [D, CB_DIM], F32, kind="ExternalInput")
    zb_d = nc.dram_tensor("zb", [CB_DIM, 1], F32, kind="ExternalInput")
    cbt_d = nc.dram_tensor("cbt", [CB_DIM + 1, CB_SIZE], F32, kind="ExternalInput")
    cbk_d = nc.dram_tensor("cbk", [CB_SIZE, CB_DIM], F32, kind="ExternalInput")

    osum_d = nc.dram_tensor("out_sum", [1, P], F32, kind="ExternalOutput")
    commit_d = nc.dram_tensor("commit", [P, 1], F32, kind="ExternalOutput")
    idx_d = nc.dram_tensor("idx_dbg", [BPC * NT, P, 1], U32, kind="ExternalOutput")

    with tile.TileContext(nc) as tc:
        with (
            tc.tile_pool(name="pp", bufs=1) as pp,
            tc.tile_pool(name="psA", bufs=2, space="PSUM") as psA,
            tc.tile_pool(name="psB", bufs=2, space="PSUM") as psB,
            tc.tile_pool(name="psC", bufs=4, space="PSUM") as psC,
        ):
            ident = pp.tile([P, P], F32, tag="ident")
            make_identity(nc, ident[:])
            nc.gpsimd.load_library(library_config.attn)
            ones_t = pp.tile([P, 1], F32, tag="ones")
            nc.vector.memset(ones_t, 1.0)
            eps_t = pp.tile([P, 1], F32, tag="eps")
            nc.vector.memset(eps_t, 1e-5)
            commit_acc = pp.tile([P, 1], F32, tag="cacc")
            nc.vector.memset(commit_acc, 0.0)
            h = [[pp.tile([P, D], F32, tag=f"h{b}_{t}", name=f"h{b}_{t}")
                  for t in range(NT)] for b in range(BPC)]
            zT = [pp.tile([CB_DIM + 1, S], F32, tag=f"zT{b}", name=f"zT{b}")
                  for b in range(BPC)]
            z_sb = [[pp.tile([P, CB_DIM], F32, tag=f"z{b}_{t}", name=f"z{b}_{t}")
                     for t in range(NT)] for b in range(BPC)]
            zb_sb = pp.tile([CB_DIM, 1], F32, tag="zb")
            nc.sync.dma_start(zb_sb, zb_d[:])

            def layer_norm(sp, xin_tiles):
                """token-major LN; returns feature-major transposed tiles."""
                xhat = []
                for t in range(NT):
                    stats = sp.tile([P, 6], F32, tag="stats", bufs=4)
                    nc.vector.bn_stats(stats, xin_tiles[t])
                    mv = sp.tile([P, 2], F32, tag="mv", bufs=4)
                    nc.vector.bn_aggr(mv, stats)
                    xc = sp.tile([P, D], F32, tag=f"xc{t}")
                    nc.vector.tensor_scalar_sub(xc, xin_tiles[t], mv[:, 0:1])
                    sqs = sp.tile([P, D], F32, tag="sqs", bufs=2)
                    ssq = sp.tile([P, 1], F32, tag="ssq", bufs=4)
                    nc.scalar.activation(sqs, xc, Act.Square, accum_out=ssq)
                    std = sp.tile([P, 1], F32, tag="std", bufs=4)
                    nc.scalar.activation(std, ssq, Act.Sqrt,
                                         bias=eps_t[:, 0:1], scale=1.0 / D)
                    r0 = sp.tile([P, 1], F32, tag="r0", bufs=4)
                    nc.vector.reciprocal(r0, std)
                    # one Newton step for 1/sqrt(a), a = ssq/D + eps
                    a_t = sp.tile([P, 1], F32, tag="a_t", bufs=4)
                    nc.vector.tensor_scalar(a_t, ssq, 1.0 / D, 1e-5,
                                            op0=Alu.mult, op1=Alu.add)
                    t1 = sp.tile([P, 1], F32, tag="t1", bufs=4)
                    nc.vector.tensor_mul(t1, a_t, r0)
                    t2 = sp.tile([P, 1], F32, tag="t2", bufs=4)
                    nc.vector.tensor_mul(t2, t1, r0)
                    u_t = sp.tile([P, 1], F32, tag="u_t", bufs=4)
                    nc.vector.tensor_scalar(u_t, t2, -0.5, 1.5,
                                            op0=Alu.mult, op1=Alu.add)
                    rstd = sp.tile([P, 1], F32, tag="rstd", bufs=4)
                    nc.vector.tensor_mul(rstd, r0, u_t)
                    nc.vector.tensor_scalar_mul(xc, xc, rstd[:, 0:1])
                    xhat.append(xc)
                xT_tiles = []
                for c in range(NC):
                    pt = psB.tile([P, S], F32, tag="B")
                    for t in range(NT):
                        nc.tensor.transpose(out=pt[:, ts(t, P)],
                                            in_=xhat[t][:, ts(c, P)],
                                            identity=ident[:])
                    xt = sp.tile([P, S], F32, tag=f"lnxT{c}", bufs=2)
                    nc.vector.tensor_copy(xt, pt)
                    xT_tiles.append(xt)
                return xT_tiles

            with (
                tc.tile_pool(name="ip", bufs=1) as ip,
                tc.tile_pool(name="ip2", bufs=2) as ip2,
            ):
                w_in_sb = ip.tile([P, NC, D], F32, tag="win")
                nc.sync.dma_start(w_in_sb,
                                  w_in_d[:].rearrange("(c p) e -> p c e", p=P))
                for b in range(BPC):
                    xT_sb = ip.tile([P, NC, S], F32, tag=f"xT{b}")
                    nc.sync.dma_start(xT_sb,
                                      xT_d[b].rearrange("(c p) t -> p c t", p=P))
                    for t in range(NT):
                        ps = psA.tile([P, D], F32, tag="A")
                        for c in range(NC):
                            nc.tensor.matmul(ps, xT_sb[:, c, ts(t, P)],
                                             w_in_sb[:, c, :],
                                             start=(c == 0), stop=(c == NC - 1))
                        pos_sb = ip2.tile([P, D], F32, tag="pos")
                        nc.sync.dma_start(pos_sb, pos_d[ts(t, P), :])
                        nc.vector.tensor_add(h[b][t], ps, pos_sb)

            with (
                tc.tile_pool(name="wp", bufs=1) as wp,
                tc.tile_pool(name="sp", bufs=1) as sp,
                tc.tile_pool(name="qk", bufs=1) as qk,
                tc.tile_pool(name="gp", bufs=2) as gp,
            ):
                for l in range(DEPTH if STAGE >= 2 else 0):
                    wq_sb = wp.tile([P, NC, D], F32, tag="wq")
                    nc.sync.dma_start(wq_sb, wq_d[l].rearrange("(c p) e -> p c e", p=P))
                    wk_sb = wp.tile([P, NC, D], F32, tag="wk")
                    nc.sync.dma_start(wk_sb, wk_d[l].rearrange("(c p) e -> p c e", p=P))
                    wv_sb = wp.tile([P, NC, D], F32, tag="wv")
                    nc.sync.dma_start(wv_sb, wv_d[l].rearrange("(c p) e -> p c e", p=P))
                    wo_sb = wp.tile([P, NC, D], F32, tag="wo")
                    nc.sync.dma_start(wo_sb, wo_d[l].rearrange("(c p) e -> p c e", p=P))
                    qb_sb = sp.tile([P, NPAIR], F32, tag="qb", bufs=2)
                    nc.sync.dma_start(qb_sb, qb_d[l])
                    kb_sb = sp.tile([P, NPAIR], F32, tag="kb", bufs=2)
                    nc.sync.dma_start(kb_sb, kb_d[l])

                    for b in range(BPC):
                        x1T = layer_norm(sp, h[b])
                        qt_t, kt_t = [], []
                        for pair in range(NPAIR):
                            psq = psA.tile([P, S], F32, tag="A")
                            for c in range(NC):
                                nc.tensor.matmul(psq, wq_sb[:, c, ts(pair, P)],
                                                 x1T[c], start=(c == 0),
                                                 stop=(c == NC - 1))
                            qt = qk.tile([P, S], F32, tag=f"qt{pair}")
                            nc.vector.tensor_scalar_add(qt, psq,
                                                        qb_sb[:, pair:pair + 1])
                            qt_t.append(qt)
                            psk = psA.tile([P, S], F32, tag="A")
                            for c in range(NC):
                                nc.tensor.matmul(psk, wk_sb[:, c, ts(pair, P)],
                                                 x1T[c], start=(c == 0),
                                                 stop=(c == NC - 1))
                            kt = qk.tile([P, S], F32, tag=f"kt{pair}")
                            nc.vector.tensor_scalar_add(kt, psk,
                                                        kb_sb[:, pair:pair + 1])
                            kt_t.append(kt)
                        vh_t = []
                        for t in range(NT):
                            psv = psA.tile([P, S], F32, tag="A")
                            for c in range(NC):
                                nc.tensor.matmul(psv, x1T[c][:, ts(t, P)],
                                                 wv_sb[:, c, :], start=(c == 0),
                                                 stop=(c == NC - 1))
                            vh = qk.tile([P, HEADS, DH + 1], F32, tag=f"vh{t}")
                            nc.vector.tensor_copy(
                                vh[:, :, 0:DH],
                                psv.rearrange("p (h e) -> p h e", h=HEADS))
                            nc.vector.memset(vh[:, :, DH:DH + 1], 1.0)
                            vh_t.append(vh)
                        ut_t = []
                        if STAGE < 3:
                            for pair in range(NPAIR):
                                ut = qk.tile([P, S], F32, tag=f"qt{pair}",
                                             name=f"utd{pair}")
                                nc.vector.memset(ut, 0.01)
                                ut_t.append(ut)
                        for head in range(HEADS if STAGE >= 3 else 0):
                            pair, half = head // 2, head % 2
                            hs = slice(half * DH, (half + 1) * DH)
                            es_t = []
                            for kc in range(NT):
                                pss = psB.tile([P, S], F32, tag="B")
                                nc.tensor.matmul(pss, kt_t[pair][hs, ts(kc, P)],
                                                 qt_t[pair][hs, :],
                                                 start=True, stop=True)
                                es = sp.tile([P, S], F32, tag=f"es{kc}", bufs=2)
                                nc.scalar.activation(es, pss, Act.Exp, scale=0.125)
                                es_t.append(es)
                            psu = psC.tile([DH + 1, S], F32, tag="C")
                            for kc in range(NT):
                                nc.tensor.matmul(psu, vh_t[kc][:, head, :],
                                                 es_t[kc], start=(kc == 0),
                                                 stop=(kc == NT - 1))
                            rd = sp.tile([1, S], F32, tag="rd", bufs=2)
                            nc.vector.reciprocal(rd, psu[DH:DH + 1, :])
                            rb = sp.tile([DH, S], F32, tag="rb", bufs=2)
                            nc.gpsimd.partition_broadcast(rb[:], rd[:])
                            if half == 0:
                                ut = qk.tile([P, S], F32, tag=f"qt{pair}")
                                ut_t.append(ut)
                            nc.vector.tensor_tensor(ut_t[pair][hs.start:hs.stop, :],
                                                    psu[0:DH, :], rb,
                                                    op=Alu.mult)
                        for t in range(NT):
                            psa = psA.tile([P, S], F32, tag="A")
                            for c in range(NC):
                                nc.tensor.matmul(psa, ut_t[c][:, ts(t, P)],
                                                 wo_sb[:, c, :], start=(c == 0),
                                                 stop=(c == NC - 1))
                            nc.vector.tensor_add(h[b][t], h[b][t], psa)

                    ff1_sb = wp.tile([P, NC, FF], F32, tag="ff1")
                    nc.sync.dma_start(ff1_sb, ff1_d[l].rearrange("(c p) e -> p c e", p=P))
                    ff1b_sb = sp.tile([P, NF], F32, tag="ff1b", bufs=2)
                    nc.sync.dma_start(ff1b_sb, ff1b_d[l])
                    ff2_sb = wp.tile([P, NF, D], F32, tag="ff2")
                    nc.sync.dma_start(ff2_sb, ff2_d[l].rearrange("(c p) e -> p c e", p=P))
                    for b in range(BPC):
                        x2T = layer_norm(sp, h[b])
                        psf2 = [psC.tile([P, D], F32, tag="C", name=f"psf2_{b}_{t2}")
                                for t2 in range(NT)]
                        for fc in range(NF):
                            psg = psA.tile([P, S], F32, tag="A")
                            for c in range(NC):
                                nc.tensor.matmul(psg, ff1_sb[:, c, ts(fc, P)],
                                                 x2T[c], start=(c == 0),
                                                 stop=(c == NC - 1))
                            g_t = gp.tile([P, S], F32, tag="g")
                            nc.scalar.activation(g_t, psg, Act.Gelu_apprx_tanh,
                                                 bias=ff1b_sb[:, fc:fc + 1])
                            for t in range(NT):
                                nc.tensor.matmul(psf2[t], g_t[:, ts(t, P)],
                                                 ff2_sb[:, fc, :],
                                                 start=(fc == 0),
                                                 stop=(fc == NF - 1))
                        for t in range(NT):
                            nc.vector.tensor_add(h[b][t], h[b][t], psf2[t])

                # final LN + projection to z
                if STAGE < 4:
                    for b in range(BPC):
                        nc.vector.memset(zT[b], 0.0)
                        nc.vector.memset(zT[b][CB_DIM:CB_DIM + 1, :], 1.0)
                        for t in range(NT):
                            nc.vector.memset(z_sb[b][t], 0.0)
                w_out_sb = sp.tile([P, NC, CB_DIM], F32, tag="wout")
                nc.sync.dma_start(w_out_sb,
                                  w_out_d[:].rearrange("(c p) e -> p c e", p=P))
                for b in range(BPC if STAGE >= 4 else 0):
                    xfT = layer_norm(sp, h[b])
                    psz = psA.tile([CB_DIM, S], F32, tag="A")
                    for c in range(NC):
                        nc.tensor.matmul(psz, w_out_sb[:, c, :], xfT[c],
                                         start=(c == 0), stop=(c == NC - 1))
                    nc.vector.tensor_scalar_add(zT[b][0:CB_DIM, :], psz,
                                                zb_sb[:, 0:1])
                    nc.vector.memset(zT[b][CB_DIM:CB_DIM + 1, :], 1.0)
                    for t in range(NT):
                        pzt = psB.tile([P, CB_DIM], F32, tag="B")
                        nc.tensor.transpose(out=pzt,
                                            in_=zT[b][0:CB_DIM, ts(t, P)],
                                            identity=ident[0:CB_DIM, 0:CB_DIM])
                        nc.vector.tensor_copy(z_sb[b][t], pzt)

            with (
                tc.tile_pool(name="vq", bufs=1) as vq,
                tc.tile_pool(name="vq2", bufs=2) as vq2,
            ):
                cbt_sb = vq.tile([CB_DIM + 1, CB_SIZE], F32, tag="cbt")
                nc.sync.dma_start(cbt_sb, cbt_d[:])
                osum_sb = vq.tile([1, P], F32, tag="osum")
                if STAGE < 5:
                    nc.vector.memset(osum_sb, 0.0)
                    nc.sync.dma_start(osum_d[:], osum_sb)
                    nc.sync.dma_start(commit_d[:], commit_acc)
                for b in range(BPC if STAGE >= 5 else 0):
                    out_ps = psC.tile([1, CB_DIM], F32, tag="C")
                    for t in range(NT):
                        score = vq2.tile([P, CB_SIZE], F32, tag="score")
                        for nk in range(NK):
                            psv = psA.tile([P, 512], F32, tag="A")
                            nc.tensor.matmul(psv, zT[b][:, ts(t, P)],
                                             cbt_sb[:, ts(nk, 512)],
                                             start=True, stop=True)
                            nc.vector.tensor_copy(score[:, ts(nk, 512)], psv)
                        t8 = vq2.tile([P, 8], F32, tag="t8")
                        i8 = vq2.tile([P, 8], U32, tag="i8")
                        if KSUB >= 2:
                            nc.vector.max(out=t8, in_=score)
                            nc.vector.max_index(out=i8, in_max=t8, in_values=score)
                            nc.sync.dma_start(idx_d[b * NT + t], i8[:, 0:1])
                        else:
                            nc.vector.memset(t8, 0.0)
                            nc.vector.memset(i8, 3)
                        q_t = vq2.tile([P, CB_DIM], F32, tag="q")
                        if KSUB >= 3:
                            nc.gpsimd.indirect_dma_start(
                                out=q_t[:], out_offset=None, in_=cbk_d[:],
                                in_offset=bass.IndirectOffsetOnAxis(ap=i8[:, 0:1],
                                                                    axis=0))
                        else:
                            nc.vector.memset(q_t, 0.5)
                        diff = vq2.tile([P, CB_DIM], F32, tag="diff")
                        nc.vector.tensor_sub(diff, q_t, z_sb[b][t])
                        sq2 = vq2.tile([P, CB_DIM], F32, tag="sq2")
                        part = vq2.tile([P, 1], F32, tag="part")
                        if KSUB >= 4:
                            nc.vector.tensor_mul(sq2, diff, diff)
                            nc.vector.tensor_reduce(part, sq2,
                                                    axis=mybir.AxisListType.X,
                                                    op=Alu.add)
                            nc.vector.tensor_add(commit_acc, commit_acc, part)
                        qst = vq2.tile([P, CB_DIM], F32, tag="qst")
                        nc.vector.tensor_add(qst, z_sb[b][t], diff)
                        if KSUB >= 5:
                            nc.tensor.matmul(out_ps, ones_t[:, 0:1], qst,
                                             start=(t == 0), stop=(t == NT - 1))
                    if KSUB >= 5:
                        nc.vector.tensor_copy(osum_sb[0:1, ts(b, CB_DIM)], out_ps)
                    else:
                        nc.vector.memset(osum_sb[0:1, ts(b, CB_DIM)], 0.0)
                nc.sync.dma_start(osum_d[:], osum_sb)
                nc.sync.dma_start(commit_d[:], commit_acc)

    nc.compile()
    return nc


def _prep_host(inputs):
    f = lambda k: np.ascontiguousarray(np.asarray(inputs[k], np.float32))
    x = f("x")
    w_in = f("w_in"); b_in = f("b_in"); pos_emb = f("pos_emb")
    ln1_g = f("ln1_g"); ln1_b = f("ln1_b")
    wq = f("wq"); wk = f("wk"); wv = f("wv"); wo = f("wo")
    ln2_g = f("ln2_g"); ln2_b = f("ln2_b")
    ff_w1 = f("ff_w1"); ff_b1 = f("ff_b1"); ff_w2 = f("ff_w2"); ff_b2 = f("ff_b2")
    lnf_g = f("lnf_g"); lnf_b = f("lnf_b")
    w_out = f("w_out"); b_out = f("b_out"); codebook = f("codebook")

    vb = np.einsum("ld,lde->le", ln1_b, wv)
    if np.any(vb) or np.any(ff_b2):
        raise NotImplementedError("nonzero value-proj/ff2 bias not supported")

    qb = np.einsum("ld,lde->le", ln1_b, wq)
    kb = np.einsum("ld,lde->le", ln1_b, wk)
    common = dict(
        w_in=w_in,
        pos_add=np.ascontiguousarray(pos_emb[:S] + b_in),
        wq=np.ascontiguousarray(wq * ln1_g[:, :, None]),
        wk=np.ascontiguousarray(wk * ln1_g[:, :, None]),
        wv=np.ascontiguousarray(wv * ln1_g[:, :, None]),
        wo=wo,
        qb=np.ascontiguousarray(qb.reshape(DEPTH, NPAIR, P).transpose(0, 2, 1)),
        kb=np.ascontiguousarray(kb.reshape(DEPTH, NPAIR, P).transpose(0, 2, 1)),
        ff_w1=np.ascontiguousarray(ff_w1 * ln2_g[:, :, None]),
        ff1b=np.ascontiguousarray(
            (np.einsum("ld,ldf->lf", ln2_b, ff_w1) + ff_b1)
            .reshape(DEPTH, NF, P).transpose(0, 2, 1)),
        ff_w2=ff_w2,
        w_out=np.ascontiguousarray(w_out * lnf_g[:, None]),
        zb=np.ascontiguousarray((lnf_b @ w_out + b_out)[:, None]),
        cbt=np.ascontiguousarray(np.concatenate(
            [codebook.T, (-0.5 * (codebook ** 2).sum(1))[None, :]], axis=0)),
        cbk=codebook,
    )
    in_maps = []
    for c in range(NCORES):
        m = dict(common)
        m["xT"] = np.ascontiguousarray(
            x[c * BPC:(c + 1) * BPC].transpose(0, 2, 1))
        in_maps.append(m)
    return in_maps


def kernel(**inputs):
    import os
    from concourse.bass_utils import run_bass_kernel_spmd
    if "nc" not in _cached:
        _cached["nc"] = _build()
    nc = _cached["nc"]
    in_maps = _prep_host(inputs)
    trace = os.environ.get("KERNEL_TRACE") == "1"
    try:
        res = run_bass_kernel_spmd(nc, in_maps, core_ids=list(range(NCORES)),
                                   trace=trace)
    except ModuleNotFoundError:
        # NTFF profiling hook unavailable on this client; run untraced
        res = run_bass_kernel_spmd(nc, in_maps, core_ids=list(range(NCORES)))
    _cached["last_res"] = res
    out = np.zeros((B, CB_DIM), np.float32)
    commit_total = np.float64(0.0)
    for c, r in enumerate(res.results):
        for b in range(BPC):
            out[c * BPC + b] = r["out_sum"][0, b * CB_DIM:(b + 1) * CB_DIM]
        commit_total += np.float64(r["commit"].sum(dtype=np.float64))
    commit = np.float32(commit_total / (B * S * CB_DIM))
    return out, commit


# revision 18
# speedup vs baseline: 1.0702x; 1.0702x over previous
"""Trainium2 Bass kernel for nn_Extractor_27290222199157.

4-layer pre-norm transformer encoder + VectorQuantize head.
Data-parallel over batch: 8 NeuronCores x 2 batch elements each.
All matmuls fp32 (argmin of the VQ is sensitive to ~1e-4 z perturbations).

Per-core layout:
  - residual h token-major: 4 tiles [128 tok, 512 d] per batch
  - LN: bn_stats mean + centered Square-accum variance + Newton-refined rstd
  - x_hat transposed to feature-major via PE transposes for the matmuls
  - attention: scores.T [k,q] per head (no max-subtract; |s|<2), exp on ACT
    (scale 1/8 fused), A@V with a ones-column appended to V so the softmax
    denominator falls out of the same matmul; normalize via DVE reciprocal +
    gpsimd partition_broadcast
  - VQ: argmax(z . cb^T - |cb|^2/2) via K=65 matmul (ones row on z),
    vector.max + max_index, indirect-DMA codebook gather,
    commit loss via tensor_tensor_reduce, token sums via ones-matmul
"""

import numpy as np

B, S, D = 16, 512, 512
HEADS, DH, DEPTH, FF = 8, 64, 4, 2048
CB_DIM, CB_SIZE = 64, 8192
NCORES = 8
BPC = B // NCORES  # batches per core
P = 128
NC = D // P    # 4 chunks of d
NT = S // P    # 4 chunks of tokens
NF = FF // P   # 16 chunks of ff
NPAIR = HEADS // 2
NK = CB_SIZE // 512  # 16 codebook column chunks

_cached = {}


def _build():
    import os
    STAGE = int(os.environ.get("KSTAGE", "5"))
    KSUB = int(os.environ.get("KSUB", "5"))
    import concourse.bass as bass
    import concourse.tile as tile
    from concourse import bacc, mybir, library_config
    from concourse.masks import make_identity

    F32 = mybir.dt.float32
    U32 = mybir.dt.uint32
    Act = mybir.ActivationFunctionType
    Alu = mybir.AluOpType

    def ts(i, n):
        return slice(i * n, (i + 1) * n)

    nc = bacc.Bacc("TRN2", target_bir_lowering=False, debug=False,
                   num_devices=NCORES)

    xT_d = nc.dram_tensor("xT", [BPC, D, S], F32, kind="ExternalInput")
    w_in_d = nc.dram_tensor("w_in", [D, D], F32, kind="ExternalInput")
    pos_d = nc.dram_tensor("pos_add", [S, D], F32, kind="ExternalInput")
    wq_d = nc.dram_tensor("wq", [DEPTH, D, D], F32, kind="ExternalInput")
    wk_d = nc.dram_tensor("wk", [DEPTH, D, D], F32, kind="ExternalInput")
    wv_d = nc.dram_tensor("wv", [DEPTH, D, D], F32, kind="ExternalInput")
    wo_d = nc.dram_tensor("wo", [DEPTH, D, D], F32, kind="ExternalInput")
    qb_d = nc.dram_tensor("qb", [DEPTH, P, NPAIR], F32, kind="ExternalInput")
    kb_d = nc.dram_tensor("kb", [DEPTH, P, NPAIR], F32, kind="ExternalInput")
    ff1_d = nc.dram_tensor("ff_w1", [DEPTH, D, FF], F32, kind="ExternalInput")
    ff1b_d = nc.dram_tensor("ff1b", [DEPTH, P, NF], F32, kind="ExternalInput")
    ff2_d = nc.dram_tensor("ff_w2", [DEPTH, FF, D], F32, kind="ExternalInput")
    w_out_d = nc.dram_tensor("w_out", [D, CB_DIM], F32, kind="ExternalInput")
    zb_d = nc.dram_tensor("zb", [CB_DIM, 1], F32, kind="ExternalInput")
    cbt_d = nc.dram_tensor("cbt", [CB_DIM + 1, CB_SIZE], F32, kind="ExternalInput")
    cbk_d = nc.dram_tensor("cbk", [CB_SIZE, CB_DIM], F32, kind="ExternalInput")

    osum_d = nc.dram_tensor("out_sum", [1, P], F32, kind="ExternalOutput")
    commit_d = nc.dram_tensor("commit", [P, 1], F32, kind="ExternalOutput")
    idx_d = nc.dram_tensor("idx_dbg", [BPC * NT, P, 1], U32, kind="ExternalOutput")

    with tile.TileContext(nc) as tc:
        with (
            tc.tile_pool(name="pp", bufs=1) as pp,
            tc.tile_pool(name="psA", bufs=2, space="PSUM") as psA,
            tc.tile_pool(name="psB", bufs=2, space="PSUM") as psB,
            tc.tile_pool(name="psC", bufs=4, space="PSUM") as psC,
        ):
            ident = pp.tile([P, P], F32, tag="ident")
            make_identity(nc, ident[:])
            nc.gpsimd.load_library(library_config.attn)
            ones_t = pp.tile([P, 1], F32, tag="ones")
            nc.vector.memset(ones_t, 1.0)
            eps_t = pp.tile([P, 1], F32, tag="eps")
            nc.vector.memset(eps_t, 1e-5)
            commit_acc = pp.tile([P, 1], F32, tag="cacc")
            nc.vector.memset(commit_acc, 0.0)
            h = [[pp.tile([P, D], F32, tag=f"h{b}_{t}", name=f"h{b}_{t}")
                  for t in range(NT)] for b in range(BPC)]
            zT = [pp.tile([CB_DIM + 1, S], F32, tag=f"zT{b}", name=f"zT{b}")
                  for b in range(BPC)]
            z_sb = [[pp.tile([P, CB_DIM], F32, tag=f"z{b}_{t}", name=f"z{b}_{t}")
                     for t in range(NT)] for b in range(BPC)]
            zb_sb = pp.tile([CB_DIM, 1], F32, tag="zb")
            nc.sync.dma_start(zb_sb, zb_d[:])

            def layer_norm(sp, xin_tiles):
                """token-major LN; returns feature-major transposed tiles."""
                xhat = []
                for t in range(NT):
                    stats = sp.tile([P, 6], F32, tag="stats", bufs=4)
                    nc.vector.bn_stats(stats, xin_tiles[t])
                    mv = sp.tile([P, 2], F32, tag="mv", bufs=4)
                    nc.vector.bn_aggr(mv, stats)
                    xc = sp.tile([P, D], F32, tag=f"xc{t}")
                    nc.vector.tensor_scalar_sub(xc, xin_tiles[t], mv[:, 0:1])
                    sqs = sp.tile([P, D], F32, tag="sqs", bufs=2)
                    ssq = sp.tile([P, 1], F32, tag="ssq", bufs=4)
                    nc.scalar.activation(sqs, xc, Act.Square, accum_out=ssq)
                    std = sp.tile([P, 1], F32, tag="std", bufs=4)
                    nc.scalar.activation(std, ssq, Act.Sqrt,
                                         bias=eps_t[:, 0:1], scale=1.0 / D)
                    r0 = sp.tile([P, 1], F32, tag="r0", bufs=4)
                    nc.vector.reciprocal(r0, std)
                    # one Newton step for 1/sqrt(a), a = ssq/D + eps
                    a_t = sp.tile([P, 1], F32, tag="a_t", bufs=4)
                    nc.vector.tensor_scalar(a_t, ssq, 1.0 / D, 1e-5,
                                            op0=Alu.mult, op1=Alu.add)
                    t1 = sp.tile([P, 1], F32, tag="t1", bufs=4)
                    nc.vector.tensor_mul(t1, a_t, r0)
                    t2 = sp.tile([P, 1], F32, tag="t2", bufs=4)
                    nc.vector.tensor_mul(t2, t1, r0)
                    u_t = sp.tile([P, 1], F32, tag="u_t", bufs=4)
                    nc.vector.tensor_scalar(u_t, t2, -0.5, 1.5,
                                            op0=Alu.mult, op1=Alu.add)
                    rstd = sp.tile([P, 1], F32, tag="rstd", bufs=4)
                    nc.vector.tensor_mul(rstd, r0, u_t)
                    nc.vector.tensor_scalar_mul(xc, xc, rstd[:, 0:1])
                    xhat.append(xc)
                xT_tiles = []
                for c in range(NC):
                    pt = psB.tile([P, S], F32, tag="B")
                    for t in range(NT):
                        nc.tensor.transpose(out=pt[:, ts(t, P)],
                                            in_=xhat[t][:, ts(c, P)],
                                            identity=ident[:])
                    xt = sp.tile([P, S], F32, tag=f"lnxT{c}", bufs=2)
                    nc.vector.tensor_copy(xt, pt)
                    xT_tiles.append(xt)
                return xT_tiles

            with (
                tc.tile_pool(name="ip", bufs=1) as ip,
                tc.tile_pool(name="ip2", bufs=2) as ip2,
            ):
                w_in_sb = ip.tile([P, NC, D], F32, tag="win")
                w_in_r = w_in_d[:].rearrange("(c p) e -> p c e", p=P)
                for c in range(NC):
                    nc.sync.dma_start(w_in_sb[:, c, :], w_in_r[:, c, :])
                for b in range(BPC):
                    xT_sb = ip.tile([P, NC, S], F32, tag=f"xT{b}")
                    xT_r = xT_d[b].rearrange("(c p) t -> p c t", p=P)
                    for c in range(NC):
                        nc.sync.dma_start(xT_sb[:, c, :], xT_r[:, c, :])
                    for t in range(NT):
                        ps = psA.tile([P, D], F32, tag="A")
                        for c in range(NC):
                            nc.tensor.matmul(ps, xT_sb[:, c, ts(t, P)],
                                             w_in_sb[:, c, :],
                                             start=(c == 0), stop=(c == NC - 1))
                        pos_sb = ip2.tile([P, D], F32, tag="pos")
                        nc.sync.dma_start(pos_sb, pos_d[ts(t, P), :])
                        nc.vector.tensor_add(h[b][t], ps, pos_sb)

            with (
                tc.tile_pool(name="wp", bufs=1) as wp,
                tc.tile_pool(name="sp", bufs=1) as sp,
                tc.tile_pool(name="qk", bufs=1) as qk,
                tc.tile_pool(name="gp", bufs=2) as gp,
            ):
                for l in range(DEPTH if STAGE >= 2 else 0):
                    wq_sb = wp.tile([P, NC, D], F32, tag="wq")
                    nc.sync.dma_start(wq_sb, wq_d[l].rearrange("(c p) e -> p c e", p=P))
                    wk_sb = wp.tile([P, NC, D], F32, tag="wk")
                    nc.sync.dma_start(wk_sb, wk_d[l].rearrange("(c p) e -> p c e", p=P))
                    wv_sb = wp.tile([P, NC, D], F32, tag="wv")
                    nc.sync.dma_start(wv_sb, wv_d[l].rearrange("(c p) e -> p c e", p=P))
                    wo_sb = wp.tile([P, NC, D], F32, tag="wo")
                    nc.sync.dma_start(wo_sb, wo_d[l].rearrange("(c p) e -> p c e", p=P))
                    qb_sb = sp.tile([P, NPAIR], F32, tag="qb", bufs=2)
                    nc.sync.dma_start(qb_sb, qb_d[l])
                    kb_sb = sp.tile([P, NPAIR], F32, tag="kb", bufs=2)
                    nc.sync.dma_start(kb_sb, kb_d[l])

                    for b in range(BPC):
                        x1T = layer_norm(sp, h[b])
                        qt_t, kt_t = [], []
                        for pair in range(NPAIR):
                            psq = psA.tile([P, S], F32, tag="A")
                            for c in range(NC):
                                nc.tensor.matmul(psq, wq_sb[:, c, ts(pair, P)],
                                                 x1T[c], start=(c == 0),
                                                 stop=(c == NC - 1))
                            qt = qk.tile([P, S], F32, tag=f"qt{pair}")
                            nc.vector.tensor_scalar_add(qt, psq,
                                                        qb_sb[:, pair:pair + 1])
                            qt_t.append(qt)
                            psk = psA.tile([P, S], F32, tag="A")
                            for c in range(NC):
                                nc.tensor.matmul(psk, wk_sb[:, c, ts(pair, P)],
                                                 x1T[c], start=(c == 0),
                                                 stop=(c == NC - 1))
                            kt = qk.tile([P, S], F32, tag=f"kt{pair}")
                            nc.vector.tensor_scalar_add(kt, psk,
                                                        kb_sb[:, pair:pair + 1])
                            kt_t.append(kt)
                        vh_t = []
                        for t in range(NT):
                            psv = psA.tile([P, S], F32, tag="A")
                            for c in range(NC):
                                nc.tensor.matmul(psv, x1T[c][:, ts(t, P)],
                                                 wv_sb[:, c, :], start=(c == 0),
                                                 stop=(c == NC - 1))
                            vh = qk.tile([P, HEADS, DH + 1], F32, tag=f"vh{t}")
                            nc.vector.tensor_copy(
                                vh[:, :, 0:DH],
                                psv.rearrange("p (h e) -> p h e", h=HEADS))
                            nc.vector.memset(vh[:, :, DH:DH + 1], 1.0)
                            vh_t.append(vh)
                        ut_t = []
                        if STAGE < 3:
                            for pair in range(NPAIR):
                                ut = qk.tile([P, S], F32, tag=f"qt{pair}",
                                             name=f"utd{pair}")
                                nc.vector.memset(ut, 0.01)
                                ut_t.append(ut)
                        for head in range(HEADS if STAGE >= 3 else 0):
                            pair, half = head // 2, head % 2
                            hs = slice(half * DH, (half + 1) * DH)
                            es_t = []
                            for kc in range(NT):
                                pss = psB.tile([P, S], F32, tag="B")
                                nc.tensor.matmul(pss, kt_t[pair][hs, ts(kc, P)],
                                                 qt_t[pair][hs, :],
                                                 start=True, stop=True)
                                es = sp.tile([P, S], F32, tag=f"es{kc}", bufs=2)
                                nc.scalar.activation(es, pss, Act.Exp, scale=0.125)
                                es_t.append(es)
                            psu = psC.tile([DH + 1, S], F32, tag="C")
                            for kc in range(NT):
                                nc.tensor.matmul(psu, vh_t[kc][:, head, :],
                                                 es_t[kc], start=(kc == 0),
                                                 stop=(kc == NT - 1))
                            rd = sp.tile([1, S], F32, tag="rd", bufs=2)
                            nc.vector.reciprocal(rd, psu[DH:DH + 1, :])
                            rb = sp.tile([DH, S], F32, tag="rb", bufs=2)
                            nc.gpsimd.partition_broadcast(rb[:], rd[:])
                            if half == 0:
                                ut = qk.tile([P, S], F32, tag=f"qt{pair}")
                                ut_t.append(ut)
                            nc.vector.tensor_tensor(ut_t[pair][hs.start:hs.stop, :],
                                                    psu[0:DH, :], rb,
                                                    op=Alu.mult)
                        for t in range(NT):
                            psa = psA.tile([P, S], F32, tag="A")
                            for c in range(NC):
                                nc.tensor.matmul(psa, ut_t[c][:, ts(t, P)],
                                                 wo_sb[:, c, :], start=(c == 0),
                                                 stop=(c == NC - 1))
                            nc.vector.tensor_add(h[b][t], h[b][t], psa)

                    ff1_sb = wp.tile([P, NC, FF], F32, tag="ff1")
                    nc.sync.dma_start(ff1_sb, ff1_d[l].rearrange("(c p) e -> p c e", p=P))
                    ff1b_sb = sp.tile([P, NF], F32, tag="ff1b", bufs=2)
                    nc.sync.dma_start(ff1b_sb, ff1b_d[l])
                    ff2_sb = wp.tile([P, NF, D], F32, tag="ff2")
                    nc.sync.dma_start(ff2_sb, ff2_d[l].rearrange("(c p) e -> p c e", p=P))
                    for b in range(BPC):
                        x2T = layer_norm(sp, h[b])
                        psf2 = [psC.tile([P, D], F32, tag="C", name=f"psf2_{b}_{t2}")
                                for t2 in range(NT)]
                        for fc in range(NF):
                            psg = psA.tile([P, S], F32, tag="A")
                            for c in range(NC):
                                nc.tensor.matmul(psg, ff1_sb[:, c, ts(fc, P)],
                                                 x2T[c], start=(c == 0),
                                                 stop=(c == NC - 1))
                            g_t = gp.tile([P, S], F32, tag="g")
                            nc.scalar.activation(g_t, psg, Act.Gelu_apprx_tanh,
                                                 bias=ff1b_sb[:, fc:fc + 1])
                            for t in range(NT):
                                nc.tensor.matmul(psf2[t], g_t[:, ts(t, P)],
                                                 ff2_sb[:, fc, :],
                                                 start=(fc == 0),
                                                 stop=(fc == NF - 1))
                        for t in range(NT):
                            nc.vector.tensor_add(h[b][t], h[b][t], psf2[t])

                # final LN + projection to z
                if STAGE < 4:
                    for b in range(BPC):
                        nc.vector.memset(zT[b], 0.0)
                        nc.vector.memset(zT[b][CB_DIM:CB_DIM + 1, :], 1.0)
                        for t in range(NT):
                            nc.vector.memset(z_sb[b][t], 0.0)
                w_out_sb = sp.tile([P, NC, CB_DIM], F32, tag="wout")
                nc.sync.dma_start(w_out_sb,
                                  w_out_d[:].rearrange("(c p) e -> p c e", p=P))
                for b in range(BPC if STAGE >= 4 else 0):
                    xfT = layer_norm(sp, h[b])
                    psz = psA.tile([CB_DIM, S], F32, tag="A")
                    for c in range(NC):
                        nc.tensor.matmul(psz, w_out_sb[:, c, :], xfT[c],
                                         start=(c == 0), stop=(c == NC - 1))
                    nc.vector.tensor_scalar_add(zT[b][0:CB_DIM, :], psz,
                                                zb_sb[:, 0:1])
                    nc.vector.memset(zT[b][CB_DIM:CB_DIM + 1, :], 1.0)
                    for t in range(NT):
                        pzt = psB.tile([P, CB_DIM], F32, tag="B")
                        nc.tensor.transpose(out=pzt,
                                            in_=zT[b][0:CB_DIM, ts(t, P)],
                                            identity=ident[0:CB_DIM, 0:CB_DIM])
                        nc.vector.tensor_copy(z_sb[b][t], pzt)

            with (
                tc.tile_pool(name="vq", bufs=1) as vq,
                tc.tile_pool(name="vq2", bufs=2) as vq2,
            ):
                cbt_sb = vq.tile([CB_DIM + 1, CB_SIZE], F32, tag="cbt")
                nc.sync.dma_start(cbt_sb, cbt_d[:])
                osum_sb = vq.tile([1, P], F32, tag="osum")
                if STAGE < 5:
                    nc.vector.memset(osum_sb, 0.0)
                    nc.sync.dma_start(osum_d[:], osum_sb)
                    nc.sync.dma_start(commit_d[:], commit_acc)
                for b in range(BPC if STAGE >= 5 else 0):
                    out_ps = psC.tile([1, CB_DIM], F32, tag="C")
                    for t in range(NT):
                        score = vq2.tile([P, CB_SIZE], F32, tag="score", bufs=3)
                        for nk in range(NK):
                            psv = psA.tile([P, 512], F32, tag="A")
                            nc.tensor.matmul(psv, zT[b][:, ts(t, P)],
                                             cbt_sb[:, ts(nk, 512)],
                                             start=True, stop=True)
                            nc.scalar.activation(score[:, ts(nk, 512)], psv,
                                                 Act.Copy)
                        t8 = vq2.tile([P, 8], F32, tag="t8")
                        i8 = vq2.tile([P, 8], U32, tag="i8")
                        if KSUB >= 2:
                            qm = []
                            for qi in range(4):
                                tq = vq2.tile([P, 8], F32, tag="t8q", bufs=4,
                                              name=f"tq{qi}")
                                nc.vector.max(out=tq,
                                              in_=score[:, ts(qi, 2048)])
                                qm.append(tq)
                            m01 = vq2.tile([P, 8], F32, tag="m01")
                            nc.vector.tensor_tensor(m01, qm[0], qm[1], op=Alu.max)
                            m23 = vq2.tile([P, 8], F32, tag="m23")
                            nc.vector.tensor_tensor(m23, qm[2], qm[3], op=Alu.max)
                            nc.vector.tensor_tensor(t8, m01, m23, op=Alu.max)
                            nc.vector.max_index(out=i8, in_max=t8, in_values=score)
                            nc.sync.dma_start(idx_d[b * NT + t], i8[:, 0:1])
                        else:
                            nc.vector.memset(t8, 0.0)
                            nc.vector.memset(i8, 3)
                        q_t = vq2.tile([P, CB_DIM], F32, tag="q")
                        if KSUB >= 3:
                            nc.gpsimd.indirect_dma_start(
                                out=q_t[:], out_offset=None, in_=cbk_d[:],
                                in_offset=bass.IndirectOffsetOnAxis(ap=i8[:, 0:1],
                                                                    axis=0))
                        else:
                            nc.vector.memset(q_t, 0.5)
                        diff = vq2.tile([P, CB_DIM], F32, tag="diff")
                        nc.vector.tensor_sub(diff, q_t, z_sb[b][t])
                        sq2 = vq2.tile([P, CB_DIM], F32, tag="sq2")
                        part = vq2.tile([P, 1], F32, tag="part")
                        if KSUB >= 4:
                            nc.vector.tensor_mul(sq2, diff, diff)
                            nc.vector.tensor_reduce(part, sq2,
                                                    axis=mybir.AxisListType.X,
                                                    op=Alu.add)
                            nc.vector.tensor_add(commit_acc, commit_acc, part)
                        qst = vq2.tile([P, CB_DIM], F32, tag="qst")
                        nc.vector.tensor_add(qst, z_sb[b][t], diff)
                        if KSUB >= 5:
                            nc.tensor.matmul(out_ps, ones_t[:, 0:1], qst,
                                             start=(t == 0), stop=(t == NT - 1))
                    if KSUB >= 5:
                        nc.vector.tensor_copy(osum_sb[0:1, ts(b, CB_DIM)], out_ps)
                    else:
                        nc.vector.memset(osum_sb[0:1, ts(b, CB_DIM)], 0.0)
                nc.sync.dma_start(osum_d[:], osum_sb)
                nc.sync.dma_start(commit_d[:], commit_acc)

    nc.compile()
    return nc


def _prep_host(inputs):
    f = lambda k: np.ascontiguousarray(np.asarray(inputs[k], np.float32))
    x = f("x")
    w_in = f("w_in"); b_in = f("b_in"); pos_emb = f("pos_emb")
    ln1_g = f("ln1_g"); ln1_b = f("ln1_b")
    wq = f("wq"); wk = f("wk"); wv = f("wv"); wo = f("wo")
    ln2_g = f("ln2_g"); ln2_b = f("ln2_b")
    ff_w1 = f("ff_w1"); ff_b1 = f("ff_b1"); ff_w2 = f("ff_w2"); ff_b2 = f("ff_b2")
    lnf_g = f("lnf_g"); lnf_b = f("lnf_b")
    w_out = f("w_out"); b_out = f("b_out"); codebook = f("codebook")

    vb = np.einsum("ld,lde->le", ln1_b, wv)
    if np.any(vb) or np.any(ff_b2):
        raise NotImplementedError("nonzero value-proj/ff2 bias not supported")

    qb = np.einsum("ld,lde->le", ln1_b, wq)
    kb = np.einsum("ld,lde->le", ln1_b, wk)
    common = dict(
        w_in=w_in,
        pos_add=np.ascontiguousarray(pos_emb[:S] + b_in),
        wq=np.ascontiguousarray(wq * ln1_g[:, :, None]),
        wk=np.ascontiguousarray(wk * ln1_g[:, :, None]),
        wv=np.ascontiguousarray(wv * ln1_g[:, :, None]),
        wo=wo,
        qb=np.ascontiguousarray(qb.reshape(DEPTH, NPAIR, P).transpose(0, 2, 1)),
        kb=np.ascontiguousarray(kb.reshape(DEPTH, NPAIR, P).transpose(0, 2, 1)),
        ff_w1=np.ascontiguousarray(ff_w1 * ln2_g[:, :, None]),
        ff1b=np.ascontiguousarray(
            (np.einsum("ld,ldf->lf", ln2_b, ff_w1) + ff_b1)
            .reshape(DEPTH, NF, P).transpose(0, 2, 1)),
        ff_w2=ff_w2,
        w_out=np.ascontiguousarray(w_out * lnf_g[:, None]),
        zb=np.ascontiguousarray((lnf_b @ w_out + b_out)[:, None]),
        cbt=np.ascontiguousarray(np.concatenate(
            [codebook.T, (-0.5 * (codebook ** 2).sum(1))[None, :]], axis=0)),
        cbk=codebook,
    )
    in_maps = []
    for c in range(NCORES):
        m = dict(common)
        m["xT"] = np.ascontiguousarray(
            x[c * BPC:(c + 1) * BPC].transpose(0, 2, 1))
        in_maps.append(m)
    return in_maps


def kernel(**inputs):
    import os
    from concourse.bass_utils import run_bass_kernel_spmd
    if "nc" not in _cached:
        _cached["nc"] = _build()
    nc = _cached["nc"]
    in_maps = _prep_host(inputs)
    trace = os.environ.get("KERNEL_TRACE") == "1"
    try:
        res = run_bass_kernel_spmd(nc, in_maps, core_ids=list(range(NCORES)),
                                   trace=trace)
    except ModuleNotFoundError:
        # NTFF profiling hook unavailable on this client; run untraced
        res = run_bass_kernel_spmd(nc, in_maps, core_ids=list(range(NCORES)))
    _cached["last_res"] = res
    out = np.zeros((B, CB_DIM), np.float32)
    commit_total = np.float64(0.0)
    for c, r in enumerate(res.results):
        for b in range(BPC):
            out[c * BPC + b] = r["out_sum"][0, b * CB_DIM:(b + 1) * CB_DIM]
        commit_total += np.float64(r["commit"].sum(dtype=np.float64))
    commit = np.float32(commit_total / (B * S * CB_DIM))
    return out, commit
